# revision 19
# baseline (speedup 1.0000x reference)
"""Bahdanau attention kernel for 8 Trainium2 NeuronCores.

Problem: B=32, S=4096, DH=EH=512, A=256 (all shapes hardcoded).
  dec_proj = decoder_hidden @ Wd                     (B, A)
  enc_proj = encoder_outputs @ We                    (B, S, A)
  energy   = tanh(dec_proj[:,None,:] + enc_proj) @ v (B, S)
  attn     = softmax(mask ? energy : -inf)           (B, S)
  context  = attn @ encoder_outputs                  (B, EH)
Returns (context, attn).

Sharding: data-parallel over batch, 4 batches per core. Weights replicated.

Device-side strategy (per core, per 512-token group g, per batch b):
  * encoder is staged from the host in BOTH layouts: encT [b, EH, S]
    (e-major, for the enc@We contraction over EH on the PE) and encN
    [b, S, EH] (natural, for the attn-weighted context contraction
    over S on the PE). This avoids all on-device transposes of the
    256 MiB tensor.
  * enc_projT[ac] (A-chunk on partitions, tokens on free) = sum over
    4 EH-chunks of We_chunk.T @ encT_chunk -> PSUM [128, 512]
  * t = tanh(enc_projT + dec_projT[:,b] as per-partition bias) on ACT,
    PSUM -> SBUF
  * energy row = sum over 2 A-chunks of v_chunk.T @ t_chunk -> PSUM
    [1, 512] placed at partition 32*b (col tile_position) so the 4
    batches share one [128, 512] PSUM tile and one 128-lane exp op.
  * w = exp(energy) * maskf (additive-max skipped: |energy| <= |v|_1,
    far from fp32 overflow; identical softmax up to rounding), with
    the row sum Z accumulated via tensor_tensor_reduce.
  * w rows are PE-transposed per 128-token chunk to put tokens on
    partitions, then used as [128, 1] lhsT to accumulate
    context = w.T @ encN into a persistent PSUM bank.
  * after all groups: attn = w * (1/Z), context = ctx_psum * (1/Z).
"""

import numpy as np

B, S, DH, EH, A = 32, 4096, 512, 512, 256
N_CORES = 8
BL = B // N_CORES  # batches per core = 4
G = 8              # token groups per batch
GS = S // G        # tokens per group = 512
NCH = S // 128     # 128-token chunks per batch = 32

# staging dtypes for the two encoder copies (np dtype objects set in _build)
ENC_T_BF16 = False  # transposed copy (energy path)
ENC_N_BF16 = False  # natural copy (context path)

# debug knobs for HW bisection (all True for the real kernel)
EN_EP = True      # enc_proj matmuls + tanh
EN_ENERGY = True  # energy matmuls + exp/mask
EN_EXP = True     # exp + mask/Z (TTR)
EN_TRANS = True   # w transposes
EN_CTX = True     # context matmuls

_cache = {}


def _build():
    import concourse.mybir as mybir
    import concourse.tile as tile
    from concourse import bacc
    from concourse.masks import make_identity

    f32 = mybir.dt.float32
    dtT = mybir.dt.bfloat16 if ENC_T_BF16 else f32
    dtN = mybir.dt.bfloat16 if ENC_N_BF16 else f32
    AF = mybir.ActivationFunctionType
    ALU = mybir.AluOpType

    nc = bacc.Bacc()

    encT = nc.declare_dram_parameter("encT", [BL, EH, S], dtT, isOutput=False)
    encN = nc.declare_dram_parameter("encN", [BL, S, EH], dtN, isOutput=False)
    maskf = nc.declare_dram_parameter("maskf", [128, S], f32, isOutput=False)
    decT = nc.declare_dram_parameter("decT", [DH, BL], f32, isOutput=False)
    Wd = nc.declare_dram_parameter("Wd", [DH, A], f32, isOutput=False)
    We = nc.declare_dram_parameter("We", [EH, A], dtT, isOutput=False)
    v2 = nc.declare_dram_parameter("v2", [A, 1], f32, isOutput=False)
    attn_out = nc.declare_dram_parameter("attn_out", [BL, S], f32, isOutput=True)
    ctx_out = nc.declare_dram_parameter("ctx_out", [BL, EH], f32, isOutput=True)

    with tile.TileContext(nc) as tc:
        with (
            tc.tile_pool(name="const", bufs=1) as const,
            tc.tile_pool(name="ld", bufs=3) as ld,
            tc.tile_pool(name="work", bufs=3) as work,
            tc.tile_pool(name="persist", bufs=1) as persist,
            tc.tile_pool(name="psum_mm", bufs=2, space="PSUM") as psum_mm,
            tc.tile_pool(name="psum_en", bufs=1, space="PSUM") as psum_en,
            tc.tile_pool(name="psum_tr", bufs=2, space="PSUM") as psum_tr,
            tc.tile_pool(name="psum_cx", bufs=1, space="PSUM") as psum_cx,
        ):
            # ---- constants ----
            We_sb = const.tile([128, 4, A], dtT)
            nc.sync.dma_start(out=We_sb, in_=We.rearrange("(c p) a -> p c a", p=128))
            Wd_sb = const.tile([128, 4, A], f32)
            nc.sync.dma_start(out=Wd_sb, in_=Wd.rearrange("(c p) a -> p c a", p=128))
            decT_sb = const.tile([128, 4, BL], f32)
            nc.sync.dma_start(out=decT_sb, in_=decT.rearrange("(c p) b -> p c b", p=128))
            v_sb = const.tile([128, 2, 1], f32)
            nc.sync.dma_start(out=v_sb, in_=v2.rearrange("(c p) o -> p c o", p=128))
            maskf_sb = persist.tile([128, S], f32)
            nc.sync.dma_start(out=maskf_sb, in_=maskf[:, :])
            ident = const.tile([128, 128], f32)
            make_identity(nc, ident)

            # ---- dec_projT [A, BL] in 2 A-chunks ----
            psum_dp = psum_en.tile([128, 2, BL], f32, tag="en")
            for ac in range(2):
                for c in range(4):
                    nc.tensor.matmul(
                        psum_dp[:, ac, :],
                        Wd_sb[:, c, ac * 128:(ac + 1) * 128],
                        decT_sb[:, c, :],
                        start=(c == 0),
                        stop=(c == 3),
                    )
            dpT_sb = const.tile([128, 2, BL], f32)
            nc.vector.tensor_copy(dpT_sb, psum_dp)

            # ---- persistent accumulators ----
            w_all = persist.tile([128, G, GS], f32)
            zparts = persist.tile([128, G], f32)
            ctx_ps = psum_cx.tile([128, EH], f32)
            nc.vector.memset(ctx_ps, 0.0)

            for g in range(G):
                encT_t = []
                encN_t = []
                for b in range(BL):
                    et = ld.tile([128, 4, GS], dtT, tag="encT", name=f"encT_{g}_{b}")
                    nc.sync.dma_start(
                        out=et,
                        in_=encT[b].rearrange("(c p) s -> p c s", p=128)[
                            :, :, g * GS:(g + 1) * GS
                        ],
                    )
                    encT_t.append(et)
                    en = ld.tile([128, 4, EH], dtN, tag="encN", name=f"encN_{g}_{b}")
                    nc.sync.dma_start(
                        out=en,
                        in_=encN[b, g * GS:(g + 1) * GS, :].rearrange(
                            "(c p) e -> p c e", p=128
                        ),
                    )
                    encN_t.append(en)

                psum_energy = psum_en.tile([128, GS], f32, tag="en",
                                           name=f"psum_energy_{g}")
                nc.vector.memset(psum_energy, 0.0)
                for b in range(BL):
                    t_sb = work.tile([128, 2, GS], f32, tag="t", name=f"t_{g}_{b}")
                    if EN_EP:
                        # enc_projT for this (b, g): [A on part, tokens free]
                        psum_ep = psum_mm.tile([128, 2, GS], f32, tag="ep",
                                               name=f"psum_ep_{g}_{b}")
                        for ac in range(2):
                            for ec in range(4):
                                nc.tensor.matmul(
                                    psum_ep[:, ac, :],
                                    We_sb[:, ec, ac * 128:(ac + 1) * 128],
                                    encT_t[b][:, ec, :],
                                    start=(ec == 0),
                                    stop=(ec == 3),
                                )
                        for ac in range(2):
                            nc.scalar.activation(
                                t_sb[:, ac, :],
                                psum_ep[:, ac, :],
                                AF.Tanh,
                                bias=dpT_sb[:, ac, b:b + 1],
                                scale=1.0,
                            )
                    else:
                        nc.vector.memset(t_sb, 0.01)
                    if EN_ENERGY:
                        for ac in range(2):
                            nc.tensor.matmul(
                                psum_energy[32 * b:32 * b + 1, :],
                                v_sb[:, ac, :],
                                t_sb[:, ac, :],
                                start=(ac == 0),
                                stop=(ac == 1),
                                tile_position=(0, 32 * b),
                            )

                # w = exp(energy) * mask; zpart = row-sum
                if EN_EXP:
                    w_raw = work.tile([128, GS], f32, tag="wr", name=f"w_raw_{g}")
                    nc.scalar.activation(w_raw, psum_energy, AF.Exp)
                    nc.vector.tensor_mul(
                        w_all[:, g, :], w_raw, maskf_sb[:, g * GS:(g + 1) * GS]
                    )
                    nc.vector.tensor_reduce(
                        zparts[:, g:g + 1], w_all[:, g, :],
                        axis=mybir.AxisListType.X, op=ALU.add,
                    )
                else:
                    nc.vector.tensor_copy(w_all[:, g, :], psum_energy)
                    nc.vector.memset(zparts[:, g:g + 1], 1.0)

                if EN_TRANS:
                    # transpose w rows -> tokens on partitions
                    psum_wT = psum_tr.tile([128, 4, 128], f32, tag="wT",
                                           name=f"psum_wT_{g}")
                    for ci in range(4):
                        nc.tensor.transpose(
                            psum_wT[:, ci, :],
                            w_all[:, g, ci * 128:(ci + 1) * 128],
                            ident,
                        )
                    wT_sb = work.tile([128, 4, 128], dtN, tag="wT", name=f"wT_{g}")
                    nc.vector.tensor_copy(wT_sb, psum_wT)

                if EN_CTX:
                    # context accumulation: ctx[32b] += w_chunk.T @ encN_chunk
                    for b in range(BL):
                        for ci in range(4):
                            nc.tensor.matmul(
                                ctx_ps[32 * b:32 * b + 1, :],
                                wT_sb[:, ci, 32 * b:32 * b + 1],
                                encN_t[b][:, ci, :],
                                start=(g == 0 and ci == 0),
                                stop=(g == G - 1 and ci == 3),
                                tile_position=(0, 32 * b),
                            )

            # ---- normalize and write outputs ----
            z = persist.tile([128, 1], f32)
            nc.vector.tensor_reduce(z, zparts, axis=mybir.AxisListType.X, op=ALU.add)
            # unused lanes sum to 0; floor them so 1/z stays finite there
            nc.vector.tensor_scalar_max(z, z, 1e-30)
            rz = persist.tile([128, 1], f32)
            nc.vector.reciprocal(rz, z)

            attn_sb = persist.tile([128, G, GS], f32)
            nc.vector.tensor_scalar_mul(attn_sb, w_all, rz)
            nc.sync.dma_start(
                out=attn_out[:, :].rearrange("b (g s) -> b g s", g=G),
                in_=attn_sb.rearrange("(b t) g s -> b t g s", b=BL)[:, 0],
            )
            ctx_sb = persist.tile([128, EH], f32)
            nc.vector.tensor_scalar_mul(ctx_sb, ctx_ps, rz)
            nc.sync.dma_start(
                out=ctx_out[:, :],
                in_=ctx_sb.rearrange("(b t) e -> b t e", b=BL)[:, 0],
            )

    nc.compile()
    return nc


def _get_nc():
    key = (ENC_T_BF16, ENC_N_BF16)
    if key not in _cache:
        _cache[key] = _build()
    return _cache[key]


def kernel(decoder_hidden, encoder_outputs, mask, Wd, We, v, _trace=False):
    import ml_dtypes
    from concourse.bass_utils import run_bass_kernel_spmd

    nc = _get_nc()

    npT = ml_dtypes.bfloat16 if ENC_T_BF16 else np.float32
    npN = ml_dtypes.bfloat16 if ENC_N_BF16 else np.float32

    decoder_hidden = np.asarray(decoder_hidden, dtype=np.float32)
    encoder_outputs = np.asarray(encoder_outputs, dtype=np.float32)
    mask = np.asarray(mask)
    Wd = np.asarray(Wd, dtype=np.float32)
    We = np.asarray(We, dtype=np.float32)
    v = np.asarray(v, dtype=np.float32)

    in_maps = []
    for c in range(N_CORES):
        sl = slice(c * BL, (c + 1) * BL)
        enc_c = encoder_outputs[sl]                      # [BL, S, EH]
        encT_c = np.ascontiguousarray(enc_c.transpose(0, 2, 1)).astype(npT)
        encN_c = np.ascontiguousarray(enc_c).astype(npN)
        maskf_c = np.zeros((128, S), dtype=np.float32)
        maskf_c[::32][:BL] = (mask[sl] != 0).astype(np.float32)
        in_maps.append({
            "encT": encT_c,
            "encN": encN_c,
            "maskf": maskf_c,
            "decT": np.ascontiguousarray(decoder_hidden[sl].T),
            "Wd": Wd,
            "We": We.astype(npT),
            "v2": v.reshape(A, 1),
        })

    res = run_bass_kernel_spmd(
        nc, in_maps, core_ids=list(range(N_CORES)), trace=_trace,
    )
    attn = np.concatenate([r["attn_out"] for r in res.results], axis=0)
    context = np.concatenate([r["ctx_out"] for r in res.results], axis=0)
    if _trace:
        kernel.last_results = res
    return context.astype(np.float32), attn.astype(np.float32)


# revision 36
# speedup vs baseline: 211.1525x; 211.1525x over previous
"""Bahdanau attention kernel for 8 Trainium2 NeuronCores.

Problem: B=32, S=4096, DH=EH=512, A=256 (all shapes hardcoded).
  dec_proj = decoder_hidden @ Wd                     (B, A)
  enc_proj = encoder_outputs @ We                    (B, S, A)
  energy   = tanh(dec_proj[:,None,:] + enc_proj) @ v (B, S)
  attn     = softmax(mask ? energy : -inf)           (B, S)
  context  = attn @ encoder_outputs                  (B, EH)
Returns (context, attn).

Sharding: data-parallel over batch, 4 batches per core, weights replicated.

Device-side strategy (per core, per 512-token group g, per batch b):
  * encoder is staged from the host in BOTH layouts: encT [b, EH, S]
    (e-major, for the enc@We contraction over EH on the PE) and encN
    [b, S, EH] (natural, for the attn-weighted context contraction over
    S on the PE). This avoids any on-device transpose of the big tensor.
  * enc_projT (A-chunks on partitions, tokens on free) = sum over 4
    EH-chunks of We_chunk.T @ encT_chunk -> PSUM [128, 2, 512]
  * t = tanh(enc_projT + dec_projT[:,b] as per-partition bias) on ACT
    (PSUM -> SBUF, bias add fused into the activation)
  * energy row = sum over 2 A-chunks of v_chunk.T @ t_chunk -> PSUM
    [1, 512] placed at partition 32*b (col tile_position), so the four
    batches share one [128, 512] PSUM tile, one 128-lane exp, and the
    four M=1 matmuls land in distinct PE column groups (concurrent).
  * w = exp(energy) * maskf. The softmax max-subtraction is skipped:
    |energy| <= |v|_1 ~ 13, far inside the fp32 exp range, and
    exp(e)*m/sum(exp(e)*m) is exactly the reference masked softmax.
  * w rows are PE-transposed per 128-token chunk to put tokens on
    partitions, then used as [128, 1] lhsT against natural encN tiles
    to accumulate unnormalized context into a persistent PSUM bank.
  * after all groups: attn = w * (1/Z), context = ctx_psum * (1/Z).

MM_DT picks the matmul dtype for the two big streams (encT/encN/We/v/t/wT):
  "f32"  - exact fp32 (PE runs fp32 at 4 cycles/row -> slowest)
  "f32r" - fp32 storage, reduced-precision single-pass matmul (1 cyc/row)
  "bf16" - bf16 storage (halves HBM traffic) + bf16 matmuls (1 cyc/row)
"""

import numpy as np

B, S, DH, EH, A = 32, 4096, 512, 512, 256
N_CORES = 8
BL = B // N_CORES  # batches per core = 4
G = 8              # token groups per batch
GS = S // G        # tokens per group = 512

MM_DT = "f32r"     # "f32" | "f32r" | "bf16"
REPEATS = 1        # repeat whole computation inside the NEFF (bench only)

_cache = {}


def _build():
    import concourse.mybir as mybir
    import concourse.tile as tile
    from concourse import bacc
    from concourse.masks import make_identity

    f32 = mybir.dt.float32
    # dt_t: encT/We (the big enc@We stream; f32r is fp32 data with a
    # reduced-precision 1-cycle/row matmul -- but f32r is rejected by the
    # ISA for col-offset tile_position, so energy/ctx matmuls stay f32)
    dt_t = {
        "f32": f32,
        "f32r": mybir.dt.float32r,
        "bf16": mybir.dt.bfloat16,
    }[MM_DT]
    # dt_n: encN/wT (context stream; per-batch PSUM banks at partition 0,
    # so f32r is legal there). v/t stay f32: the energy matmuls use col
    # tile_position, which the ISA rejects for f32r.
    dt_n = dt_t
    dt_e = mybir.dt.bfloat16 if MM_DT == "bf16" else f32
    AF = mybir.ActivationFunctionType
    ALU = mybir.AluOpType

    nc = bacc.Bacc()

    encT = nc.declare_dram_parameter("encT", [G, 128, BL, 4, GS], dt_t,
                                 isOutput=False)
    encN = nc.declare_dram_parameter("encN", [G, 128, BL, 4, EH], dt_n,
                                 isOutput=False)
    maskf = nc.declare_dram_parameter("maskf", [BL, S], f32, isOutput=False)
    decT = nc.declare_dram_parameter("decT", [DH, BL], f32, isOutput=False)
    Wd = nc.declare_dram_parameter("Wd", [DH, A], f32, isOutput=False)
    We = nc.declare_dram_parameter("We", [EH, A], dt_t, isOutput=False)
    v2 = nc.declare_dram_parameter("v2", [A, 1], dt_e, isOutput=False)
    attn_out = nc.declare_dram_parameter("attn_out", [BL, S], f32, isOutput=True)
    ctx_out = nc.declare_dram_parameter("ctx_out", [BL, EH], f32, isOutput=True)

    with tile.TileContext(nc) as tc:
        with (
            tc.tile_pool(name="const", bufs=1) as const,
            tc.tile_pool(name="ld", bufs=3) as ld,
            tc.tile_pool(name="work", bufs=3) as work,
            tc.tile_pool(name="persist", bufs=1) as persist,
            tc.tile_pool(name="psum_mm", bufs=2, space="PSUM") as psum_mm,
            tc.tile_pool(name="psum_en", bufs=1, space="PSUM") as psum_en,
            tc.tile_pool(name="psum_tr", bufs=1, space="PSUM") as psum_tr,
            tc.tile_pool(name="psum_cx", bufs=1, space="PSUM") as psum_cx,
        ):
            # ---- constants ----
            We_sb = const.tile([128, 4, A], dt_t)
            nc.sync.dma_start(out=We_sb, in_=We.rearrange("(c p) a -> p c a", p=128))
            Wd_sb = const.tile([128, 4, A], f32)
            nc.sync.dma_start(out=Wd_sb, in_=Wd.rearrange("(c p) a -> p c a", p=128))
            decT_sb = const.tile([128, 4, BL], f32)
            nc.sync.dma_start(out=decT_sb,
                              in_=decT.rearrange("(c p) b -> p c b", p=128))
            v_sb = const.tile([128, 2, 1], dt_e)
            nc.sync.dma_start(out=v_sb, in_=v2.rearrange("(c p) o -> p c o", p=128))
            maskf_sb = persist.tile([128, S], f32, tag="bigrow")
            nc.vector.memset(maskf_sb, 0.0)
            nc.sync.dma_start(
                out=maskf_sb.rearrange("(b t) s -> b t s", b=BL)[:, 0],
                in_=maskf[:, :],
            )
            ident = const.tile([128, 128], f32)
            make_identity(nc, ident)

            # ---- dec_projT [A, BL] in 2 A-chunks ----
            psum_dp = psum_en.tile([128, 2, BL], f32, tag="en")
            for ac in range(2):
                for c in range(4):
                    nc.tensor.matmul(
                        psum_dp[:, ac, :],
                        Wd_sb[:, c, ac * 128:(ac + 1) * 128],
                        decT_sb[:, c, :],
                        start=(c == 0),
                        stop=(c == 3),
                    )
            dpT_sb = const.tile([128, 2, BL], f32)
            nc.vector.tensor_copy(dpT_sb, psum_dp)

            for _rep in range(REPEATS):
                w_all = persist.tile([128, G, GS], f32)
                zparts = persist.tile([128, G], f32)
                # one PSUM bank per batch so the f32r context matmuls write
                # at partition 0 (f32r + col tile_position is rejected)
                ctx_ps = [
                    psum_cx.tile([1, EH], f32, tag=f"cx{b}", name=f"ctx_ps_{b}")
                    for b in range(BL)
                ]

                for g in range(G):
                    # host stages the data in the exact tile layout, so
                    # each per-batch load is one fully-contiguous 1 MiB DMA
                    encT_t = []
                    encN_t = []
                    for b in range(BL):
                        et = ld.tile([128, 4, GS], dt_t, tag="encT", bufs=6,
                                     name=f"encT_{g}_{b}")
                        nc.sync.dma_start(out=et, in_=encT[g, :, b])
                        encT_t.append(et)
                        en = ld.tile([128, 4, EH], dt_n, tag="encN", bufs=6,
                                     name=f"encN_{g}_{b}")
                        nc.sync.dma_start(out=en, in_=encN[g, :, b])
                        encN_t.append(en)

                    psum_energy = psum_en.tile([128, GS], f32, tag="en",
                                               name=f"psum_energy_{g}")
                    nc.vector.memset(psum_energy, 0.0)

                    t_all = []
                    for b in range(BL):
                        t_sb = work.tile([128, 2, GS], dt_e, tag="t", bufs=6,
                                         name=f"t_{g}_{b}")
                        for ac in range(2):
                            psum_ep = psum_mm.tile([128, GS], f32, tag="ep",
                                                   name=f"psum_ep_{g}_{b}_{ac}")
                            for ec in range(4):
                                nc.tensor.matmul(
                                    psum_ep,
                                    We_sb[:, ec, ac * 128:(ac + 1) * 128],
                                    encT_t[b][:, ec, :],
                                    start=(ec == 0),
                                    stop=(ec == 3),
                                )
                            nc.scalar.activation(
                                t_sb[:, ac, :],
                                psum_ep,
                                AF.Tanh,
                                bias=dpT_sb[:, ac, b:b + 1],
                                scale=1.0,
                            )
                        t_all.append(t_sb)

                    # energy: ac-outer so the 4 batches' M=1 matmuls hit 4
                    # distinct PE column groups back-to-back (concurrent)
                    for ac in range(2):
                        for b in range(BL):
                            nc.tensor.matmul(
                                psum_energy[32 * b:32 * b + 1, :],
                                v_sb[:, ac, :],
                                t_all[b][:, ac, :],
                                start=(ac == 0),
                                stop=(ac == 1),
                                tile_position=(0, 32 * b),
                            )

                    # w = exp(energy) * mask; zpart = row-sum
                    w_raw = work.tile([128, GS], f32, tag="wr", name=f"w_raw_{g}")
                    nc.scalar.activation(w_raw, psum_energy, AF.Exp)
                    nc.vector.tensor_mul(
                        w_all[:, g, :], w_raw, maskf_sb[:, g * GS:(g + 1) * GS]
                    )
                    nc.vector.tensor_reduce(
                        zparts[:, g:g + 1], w_all[:, g, :],
                        axis=mybir.AxisListType.X, op=ALU.add,
                    )

                    # transpose w rows -> tokens on partitions
                    psum_wT = psum_tr.tile([128, 4, 128], f32, tag="wT",
                                           name=f"psum_wT_{g}")
                    for ci in range(4):
                        nc.tensor.transpose(
                            psum_wT[:, ci, :],
                            w_all[:, g, ci * 128:(ci + 1) * 128],
                            ident,
                        )
                    wT_sb = work.tile([128, 4, 128], dt_n, tag="wTs",
                                      name=f"wT_{g}")
                    nc.vector.tensor_copy(wT_sb, psum_wT)

                    # context accumulation (f32r, per-batch PSUM banks)
                    for ci in range(4):
                        for b in range(BL):
                            nc.tensor.matmul(
                                ctx_ps[b][:, :],
                                wT_sb[:, ci, 32 * b:32 * b + 1],
                                encN_t[b][:, ci, :],
                                start=(g == 0 and ci == 0),
                                stop=(g == G - 1 and ci == 3),
                            )

                # ---- normalize and write outputs ----
                z = persist.tile([128, 1], f32)
                nc.vector.tensor_reduce(z, zparts, axis=mybir.AxisListType.X,
                                        op=ALU.add)
                # unused lanes sum to 0; floor them so 1/z stays finite there
                nc.vector.tensor_scalar_max(z, z, 1e-30)
                rz = persist.tile([128, 1], f32)
                nc.vector.reciprocal(rz, z)

                # share the maskf slot when possible (REPEATS>1 still needs
                # maskf alive in later reps, so keep a separate slot then)
                attn_sb = persist.tile([128, G, GS], f32,
                                       tag="bigrow" if REPEATS == 1 else "attn")
                nc.vector.tensor_scalar_mul(attn_sb, w_all, rz)
                nc.sync.dma_start(
                    out=attn_out[:, :].rearrange("b (g s) -> b g s", g=G),
                    in_=attn_sb.rearrange("(b t) g s -> b t g s", b=BL)[:, 0],
                )
                ctx_sb = persist.tile([128, EH], f32)
                nc.vector.memset(ctx_sb, 0.0)
                for b in range(BL):
                    nc.vector.tensor_copy(ctx_sb[32 * b:32 * b + 1, :], ctx_ps[b])
                ctx_sc = persist.tile([128, EH], f32)
                nc.vector.tensor_scalar_mul(ctx_sc, ctx_sb, rz)
                nc.sync.dma_start(
                    out=ctx_out[:, :],
                    in_=ctx_sc.rearrange("(b t) e -> b t e", b=BL)[:, 0],
                )

    nc.compile()
    return nc


def _get_nc():
    key = (MM_DT, REPEATS)
    if key not in _cache:
        _cache[key] = _build()
    return _cache[key]


def _np_dts():
    """(np dtype for encT/We/encN, np dtype for v2)"""
    if MM_DT == "bf16":
        import ml_dtypes
        return ml_dtypes.bfloat16, ml_dtypes.bfloat16
    return np.float32, np.float32


def kernel(decoder_hidden, encoder_outputs, mask, Wd, We, v, _trace=False):
    from concourse.bass_utils import run_bass_kernel_spmd

    nc = _get_nc()
    npd_t, npd_n = _np_dts()

    decoder_hidden = np.asarray(decoder_hidden, dtype=np.float32)
    encoder_outputs = np.asarray(encoder_outputs, dtype=np.float32)
    mask = np.asarray(mask)
    Wd = np.asarray(Wd, dtype=np.float32)
    We = np.asarray(We, dtype=np.float32)
    v = np.asarray(v, dtype=np.float32)

    in_maps = []
    for c in range(N_CORES):
        sl = slice(c * BL, (c + 1) * BL)
        enc_c = encoder_outputs[sl]                      # [BL, S, EH]
        maskf_c = (mask[sl] != 0).astype(np.float32)
        in_maps.append({
            # device tile layouts: encT[g,p,b,c,s] = enc[b, g*GS+s, c*128+p]
            #                       encN[g,p,b,c,e] = enc[b, (g*4+c)*128+p, e]
            "encT": np.ascontiguousarray(
                enc_c.reshape(BL, G, GS, 4, 128).transpose(1, 4, 0, 3, 2)
            ).astype(npd_t),
            "encN": np.ascontiguousarray(
                enc_c.reshape(BL, G, 4, 128, EH).transpose(1, 3, 0, 2, 4)
            ).astype(npd_n),
            "maskf": maskf_c,
            "decT": np.ascontiguousarray(decoder_hidden[sl].T),
            "Wd": Wd,
            "We": We.astype(npd_t),
            "v2": v.reshape(A, 1).astype(npd_n),
        })

    # the axon-tunneled devices intermittently report
    # NRT_EXEC_UNIT_UNRECOVERABLE on a fresh first execution and recover on
    # retry; retry a few times before giving up
    import time as _time
    last_exc = None
    for attempt in range(4):
        try:
            res = run_bass_kernel_spmd(
                nc, in_maps, core_ids=list(range(N_CORES)), trace=_trace,
            )
            break
        except Exception as e:  # noqa: BLE001
            last_exc = e
            _time.sleep(10 * (attempt + 1))
    else:
        raise last_exc
    attn = np.concatenate([r["attn_out"] for r in res.results], axis=0)
    context = np.concatenate([r["ctx_out"] for r in res.results], axis=0)
    if _trace:
        kernel.last_results = res
    return context.astype(np.float32), attn.astype(np.float32)
